# revision 4
# baseline (speedup 1.0000x reference)
"""BRF (bursting resonate-and-fire) neuron update kernel for Trainium2.

Computes, elementwise over [B=4096, D=4096] fp32 tensors (per-neuron
vectors omegas/bs/threshold along D):

    omega  = |omegas|
    p      = (-1 + sqrt(1 - (DT*omega)^2)) / DT
    b      = p - |bs| - q
    u_     = u + b*u*DT - omega*v*DT + x*DT
    v_new  = v + omega*u*DT + b*v*DT
    z      = heaviside(|u_| - |threshold| - q)
    q_new  = q*0.9 + z

Layout: TRANSPOSED — neurons (D) on SBUF partitions, batch (B) on the free
dim. The D axis is sharded across the 8 cores (512 neurons each); the host
hands each core contiguous [512, 4096] transposed slabs. Per-neuron
constants then live as per-partition scalars ([128, n_pb] f32), so no
broadcast DMAs are needed and tensor_scalar ops fold them in for free.

Mixed precision (rel-err budget 2e-2, measured worst ~8e-3):
  - u, q stay fp32 (the spike margin |u_|-|th|-q needs ~1e-5 accuracy).
  - v and xs=DT*x are uploaded bf16; the correction
        du = a*u - W*v + xs,  a = a0 - DT*q  (|du| ~ 1e-4..4e-3)
    is accumulated in bf16 (error ~1e-5 of du => ~1e-8 absolute).
  - u_ = u + du and the spike compare run in fp32.
  - Outputs: u_, v_, q_ stored bf16; z stored u8. Host converts to f32.
"""

import os

import numpy as np

DT = 1.0 / 24000.0
Q_DECAY = 0.9
B, D = 4096, 4096
N_CORES = 8
COLS = D // N_CORES  # neurons per core (partition-dim rows of the slab)
P = 128  # SBUF partitions

# Set by kernel() after a run: ns of the slowest core (None if profiling
# unavailable through this client).
LAST_EXEC_TIME_NS = None
LAST_RESULTS = None


def _legalize_bir_waits(raw: bytes) -> bytes:
    """Split multi-wait instructions into EventSemaphore + 1-wait instruction.

    The walrus build in this toolchain encodes at most ONE sync-wait per
    instruction; Tile's semaphore assignment emits several. Hoisting the
    extra waits onto standalone EventSemaphore instructions immediately
    before the instruction (same engine stream, in-order) is semantically
    identical.
    """
    import json

    d = json.loads(raw)
    for fn in d.get("functions", []):
        for bb in fn.get("blocks", []):
            out = []
            for ins in bb.get("instructions", []):
                si = ins.get("sync_info") or {}
                waits = si.get("on_wait") or []
                if len(waits) > 1:
                    for k, w in enumerate(waits[:-1]):
                        out.append(
                            {
                                "debug": ins.get("debug", 0),
                                "engine": ins["engine"],
                                "ins": [],
                                "name": f"{ins['name']}-w{k}",
                                "opcode": "EventSemaphore",
                                "outs": [],
                                "sync_info": {"on_update": [], "on_wait": [w]},
                            }
                        )
                    si["on_wait"] = [waits[-1]]
                out.append(ins)
            bb["instructions"] = out
    return json.dumps(d).encode()


def _install_wait_legalizer(nc):
    orig = nc.to_json_bytes

    def patched():
        return _legalize_bir_waits(orig())

    nc.to_json_bytes = patched
    return nc


def build_nc(rows=COLS, b=B, free=2048, repeat=1, dma_only=False,
             io_bufs=3, out_bufs=2, tmp_bufs=2, eng=None):
    """Per-core Bass program (identical on all 8 cores), transposed layout.

    rows: neurons on this core (partition dim, 512 = 4 blocks of 128).
    b:    batch size (free dim, chunked by `free`).
    repeat > 1 re-emits the whole main loop (same work and DRAM traffic
    each pass) — for slope-based timing. dma_only skips compute and stores
    loaded bytes back (same DMA traffic) — the pure memory floor.
    eng:  dict op-name -> engine ("v"=DVE, "p"=Pool) for A/B tuning.
    """
    import concourse.bass as bass
    import concourse.mybir as mybir
    from concourse.tile import TileContext

    f32 = mybir.dt.float32
    bf = mybir.dt.bfloat16
    u8 = mybir.dt.uint8
    Alu = mybir.AluOpType
    Act = mybir.ActivationFunctionType

    # default engine split: DVE gets the TS/bf16-fast ops, Pool the rest
    # Pool rejects TensorScalarPtr at codegen, so all tensor_scalar /
    # scalar_tensor_tensor ops live on DVE; Pool takes five plain TTs.
    E = {
        "a": "v", "p2": "v", "p4": "v", "thq": "v", "q_": "v", "z": "v",
        "d3": "v", "v_": "v",
        "t1": "p", "d1": "p", "d2": "p", "u_": "p", "t2": "p",
    }
    if eng:
        E.update(eng)

    nc = bass.Bass(trn_type="TRN2")

    u = nc.dram_tensor("u", [rows, b], f32, kind="ExternalInput")
    q = nc.dram_tensor("q", [rows, b], f32, kind="ExternalInput")
    v = nc.dram_tensor("v", [rows, b], bf, kind="ExternalInput")
    xs = nc.dram_tensor("xs", [rows, b], bf, kind="ExternalInput")
    n_pb = rows // P
    a0s = nc.dram_tensor("a0s", [P, n_pb], f32, kind="ExternalInput")
    ws = nc.dram_tensor("ws", [P, n_pb], f32, kind="ExternalInput")
    ths = nc.dram_tensor("ths", [P, n_pb], f32, kind="ExternalInput")

    z_o = nc.dram_tensor("z_o", [rows, b], u8, kind="ExternalOutput")
    u_o = nc.dram_tensor("u_o", [rows, b], bf, kind="ExternalOutput")
    v_o = nc.dram_tensor("v_o", [rows, b], bf, kind="ExternalOutput")
    q_o = nc.dram_tensor("q_o", [rows, b], bf, kind="ExternalOutput")

    n_fc = b // free

    with TileContext(nc) as tc:
        with (
            tc.tile_pool(name="consts", bufs=1) as cp,
            tc.tile_pool(name="io", bufs=io_bufs) as iop,
            tc.tile_pool(name="out", bufs=out_bufs) as op_,
            tc.tile_pool(name="tmp", bufs=tmp_bufs) as tp,
        ):
            a0t = cp.tile([P, n_pb], f32, tag="a0")
            wt = cp.tile([P, n_pb], f32, tag="w")
            tht = cp.tile([P, n_pb], f32, tag="th")
            nc.sync.dma_start(out=a0t[:], in_=a0s[:, :])
            nc.sync.dma_start(out=wt[:], in_=ws[:, :])
            nc.sync.dma_start(out=tht[:], in_=ths[:, :])

            def engine(name):
                return nc.vector if E[name] == "v" else nc.gpsimd

            for it in range(n_pb * n_fc * repeat):
                pb = (it // n_fc) % n_pb
                fc = it % n_fc
                rs = slice(pb * P, (pb + 1) * P)
                cs = slice(fc * free, (fc + 1) * free)

                ut = iop.tile([P, free], f32, tag="u")
                qt = iop.tile([P, free], f32, tag="q")
                vt = iop.tile([P, free], bf, tag="v")
                xt = iop.tile([P, free], bf, tag="xs")
                nc.sync.dma_start(out=ut[:], in_=u[rs, cs])
                nc.sync.dma_start(out=qt[:], in_=q[rs, cs])
                nc.sync.dma_start(out=vt[:], in_=v[rs, cs])
                nc.sync.dma_start(out=xt[:], in_=xs[rs, cs])

                if dma_only:
                    ub = op_.tile([P, free], bf, tag="ub")
                    vb = op_.tile([P, free], bf, tag="vb")
                    qb = op_.tile([P, free], bf, tag="qb")
                    zt = op_.tile([P, free], u8, tag="z")
                    nc.vector.tensor_scalar(ub[:], vt[:], 1.0, None, Alu.mult)
                    nc.vector.tensor_scalar(vb[:], xt[:], 1.0, None, Alu.mult)
                    nc.vector.tensor_scalar(qb[:], vt[:], 1.0, None, Alu.mult)
                    nc.vector.memset(zt[:], 0)
                    nc.scalar.dma_start(out=u_o[rs, cs], in_=ub[:])
                    nc.scalar.dma_start(out=v_o[rs, cs], in_=vb[:])
                    nc.scalar.dma_start(out=q_o[rs, cs], in_=qb[:])
                    nc.scalar.dma_start(out=z_o[rs, cs], in_=zt[:])
                    continue

                a0c = a0t[:, pb : pb + 1]
                wc = wt[:, pb : pb + 1]
                thc = tht[:, pb : pb + 1]

                # a = a0 - DT*q            (bf16; a = DT*b of the reference)
                at = tp.tile([P, free], bf, tag="a")
                engine("a").tensor_scalar(at[:], qt[:], -DT, a0c, Alu.mult, Alu.add)
                # p2 = W*v                 (bf16 4x TS)
                p2 = tp.tile([P, free], bf, tag="p2")
                engine("p2").tensor_scalar(p2[:], vt[:], wc, None, Alu.mult)
                # t1 = a*u                 (mixed; Pool)
                t1 = tp.tile([P, free], bf, tag="t1")
                engine("t1").tensor_tensor(t1[:], at[:], ut[:], Alu.mult)
                # d1 = t1 - p2; d2 = d1 + xs   (bf16 TT)
                d1 = tp.tile([P, free], bf, tag="d1")
                engine("d1").tensor_tensor(d1[:], t1[:], p2[:], Alu.subtract)
                d2 = tp.tile([P, free], bf, tag="t1")
                engine("d2").tensor_tensor(d2[:], d1[:], xt[:], Alu.add)
                # u_ = u + d2              (fp32; spike-critical)
                uf = tp.tile([P, free], f32, tag="uf")
                engine("u_").tensor_tensor(uf[:], ut[:], d2[:], Alu.add)
                # p4 = W*u                 (f32-in TS)
                p4 = tp.tile([P, free], bf, tag="p4")
                engine("p4").tensor_scalar(p4[:], ut[:], wc, None, Alu.mult)
                # t2 = a*v; d3 = t2 + p4; v_ = v + d3   (bf16 TT)
                t2 = tp.tile([P, free], bf, tag="p2")
                engine("t2").tensor_tensor(t2[:], at[:], vt[:], Alu.mult)
                d3 = tp.tile([P, free], bf, tag="d1")
                engine("d3").tensor_tensor(d3[:], t2[:], p4[:], Alu.add)
                vb = op_.tile([P, free], bf, tag="vb")
                engine("v_").tensor_tensor(vb[:], vt[:], d3[:], Alu.add)
                # thq = q + TH             (fp32 TS)
                thq = tp.tile([P, free], f32, tag="thq")
                engine("thq").tensor_scalar(thq[:], qt[:], thc, None, Alu.add)
                # au = |u_|                (ACT)
                au = tp.tile([P, free], f32, tag="au")
                nc.scalar.activation(au[:], uf[:], Act.Abs)
                # z = au > thq             (u8)
                zt = op_.tile([P, free], u8, tag="z")
                engine("z").tensor_tensor(zt[:], au[:], thq[:], Alu.is_gt)
                # q_ = 0.9*q + z           (bf16 out)
                qb = op_.tile([P, free], bf, tag="qb")
                engine("q_").scalar_tensor_tensor(
                    qb[:], qt[:], Q_DECAY, zt[:], Alu.mult, Alu.add
                )
                # u_b = bf16(u_)           (ACT copy-cast)
                ub = op_.tile([P, free], bf, tag="ub")
                nc.scalar.activation(ub[:], uf[:], Act.Copy)

                nc.scalar.dma_start(out=u_o[rs, cs], in_=ub[:])
                nc.scalar.dma_start(out=v_o[rs, cs], in_=vb[:])
                nc.scalar.dma_start(out=q_o[rs, cs], in_=qb[:])
                nc.scalar.dma_start(out=z_o[rs, cs], in_=zt[:])

    return _install_wait_legalizer(nc)


def host_prep(x, u, v, q, omegas, bs, threshold):
    """Fold per-neuron vectors; build per-core transposed input slabs."""
    import ml_dtypes

    f = np.float32
    bf = ml_dtypes.bfloat16

    om = np.abs(np.asarray(omegas, dtype=f))
    w = (f(DT) * om).astype(f)
    p = ((f(-1.0) + np.sqrt((f(1.0) - w * w).astype(f))) / f(DT)).astype(f)
    a0 = (f(DT) * (p - np.abs(np.asarray(bs, dtype=f)))).astype(f)
    th = np.abs(np.asarray(threshold, dtype=f))

    x = np.asarray(x, dtype=f)
    u = np.asarray(u, dtype=f)
    v = np.asarray(v, dtype=f)
    q = np.asarray(q, dtype=f)
    xs = (x * f(DT)).astype(f)

    n_pb = COLS // P
    in_maps = []
    for k in range(N_CORES):
        sl = slice(k * COLS, (k + 1) * COLS)
        in_maps.append(
            {
                "u": np.ascontiguousarray(u[:, sl].T),
                "q": np.ascontiguousarray(q[:, sl].T),
                "v": np.ascontiguousarray(v[:, sl].T).astype(bf),
                "xs": np.ascontiguousarray(xs[:, sl].T).astype(bf),
                "a0s": np.ascontiguousarray(a0[sl].reshape(n_pb, P).T),
                "ws": np.ascontiguousarray(w[sl].reshape(n_pb, P).T),
                "ths": np.ascontiguousarray(th[sl].reshape(n_pb, P).T),
            }
        )
    return in_maps


_NC_CACHE = {}


def kernel(x, u, v, q, omegas, bs, threshold):
    global LAST_EXEC_TIME_NS, LAST_RESULTS
    from concourse import bass_utils

    key = "nc"
    if key not in _NC_CACHE:
        _NC_CACHE[key] = build_nc()
    nc = _NC_CACHE[key]

    in_maps = host_prep(x, u, v, q, omegas, bs, threshold)

    trace = bool(int(os.environ.get("BRF_TRACE", "0")))
    res = bass_utils.run_bass_kernel_spmd(
        nc, in_maps, core_ids=list(range(N_CORES)), trace=trace
    )
    LAST_EXEC_TIME_NS = res.exec_time_ns
    LAST_RESULTS = res

    f = np.float32
    zf = np.empty((B, D), dtype=f)
    uf = np.empty((B, D), dtype=f)
    vf = np.empty((B, D), dtype=f)
    qf = np.empty((B, D), dtype=f)
    for k in range(N_CORES):
        sl = slice(k * COLS, (k + 1) * COLS)
        r = res.results[k]
        zf[:, sl] = r["z_o"].T
        uf[:, sl] = r["u_o"].astype(f).T
        vf[:, sl] = r["v_o"].astype(f).T
        qf[:, sl] = r["q_o"].astype(f).T
    return (zf, uf, vf, qf)
